# revision 3
# baseline (speedup 1.0000x reference)
"""Trainium2 Bass kernel for nn_ClusteringLayer (vq_codebook).

Computes, for x (B,D) and clusters (K,D):
    sq   = ||x_i||^2 - 2 x.clusters^T + ||c_j||^2     (B,K)
    dist = sqrt(sq)
    num  = 1 / (1 + dist)          (ALPHA=1 -> exponent -1)
    out  = num / sum(num)          (global scalar normalizer)

Sharding: data-parallel on batch across 8 NeuronCores; clusters
replicated; one 4-byte AllReduce for the normalizer.

Host-side prep is layout/precision only: x/clusters are passed
transposed (d-major) so the contraction dim lands on SBUF partitions
with no on-chip transposes, and pre-rounded to fp32r (E8M11,
round-to-nearest-even) — the TensorEngine's fast-fp32 mode operates at
that precision anyway; rounding on host is strictly more accurate than
the on-device truncation and satisfies walrus' "operand must be fp32r-
rounded" producer check.

Per-core device program (Bl = B/8 = 2048 local rows):
  - load xT (D,Bl) and cT (D,K) fp32r as 4 partition-chunks each
  - xsq/csq via ACT Square (fp32r out)
  - x2 per-row sums of squares via N=2 ones-matmuls -> (128, 2*MT)
    (fp32r matmuls reject N=1); c2 via M=1 ones-matmuls -> (1,K) row,
    copied with scale -0.5 so psum = x.c^T - c2/2
  - per (128,512) tile: 4 accumulating fp32r matmuls + 1 K=1 fold
    matmul (adds -c2/2), then one ACT pass Sqrt(-2*psum + x2) -> dist
  - dist+1 (DVE tensor_scalar), reciprocal_approx_fast (DVE) -> num
  - global sum: DVE cast slices to fp32r, ones-matmul reductions into
    one PSUM row, DVE reduce -> 4-byte AllReduce -> reciprocal ->
    K=1 fp32 matmul broadcast to (128,1)
  - DVE per-partition scale, DMA out
"""

import numpy as np

B, D, K = 16384, 512, 1024
N_CORES = 8
BL = B // N_CORES        # 2048 rows per core
P = 128                  # partitions
MT = BL // P             # 16 m-tiles per core
KC = D // P              # 4 contraction chunks
NJ = 512                 # matmul moving free dim (fp32 max)
JH = K // NJ             # 2 j-halves
NSL = 8                  # elementwise slices
SW = MT * K // NSL       # 2048 free elems per slice

_CACHE = {}


def _round_f32r(a: np.ndarray) -> np.ndarray:
    """Round fp32 to fp32r (E8M11): round-to-nearest-even on the low 12
    mantissa bits. Inputs here are gaussian draws — no inf/nan."""
    bits = np.ascontiguousarray(a, dtype=np.float32).view(np.uint32)
    lsb = (bits >> np.uint32(12)) & np.uint32(1)
    out = (bits + np.uint32(0x7FF) + lsb) & np.uint32(0xFFFFF000)
    return out.view(np.float32)


def _build_bass():
    import concourse.bass as bass  # noqa: F401
    import concourse.mybir as mybir
    import concourse.tile as tile
    from concourse import bacc

    f32 = mybir.dt.float32
    f32r = mybir.dt.float32r
    AF = mybir.ActivationFunctionType

    nc = bacc.Bacc(
        "TRN2", target_bir_lowering=False, debug=False, num_devices=N_CORES
    )
    xT_d = nc.dram_tensor("xT", [D, BL], f32r, kind="ExternalInput").ap()
    cT_d = nc.dram_tensor("cT", [D, K], f32r, kind="ExternalInput").ap()
    out_d = nc.dram_tensor("out", [BL, K], f32, kind="ExternalOutput").ap()

    with tile.TileContext(nc) as tc:
        with (
            tc.tile_pool(name="const", bufs=1) as cpool,
            tc.tile_pool(name="big", bufs=1) as bpool,
            tc.tile_pool(name="sq", bufs=KC) as sqpool,
            tc.tile_pool(name="numr", bufs=2) as nrpool,
            tc.tile_pool(name="pprep", bufs=1, space="PSUM") as pprep,
            tc.tile_pool(name="pmm", bufs=3, space="PSUM") as pmm,
            tc.tile_pool(name="dram", bufs=1, space="DRAM") as dpool,
        ):
            ones_col_f = cpool.tile([P, 2], f32)
            nc.gpsimd.memset(ones_col_f, 1.0)
            ones_col = cpool.tile([P, 2], f32r)  # [:, :1] = M=1 lhsT
            nc.vector.tensor_copy(ones_col, ones_col_f)
            ones_row_f = cpool.tile([1, P], f32)  # fp32 lhsT for inv bcast
            nc.gpsimd.memset(ones_row_f, 1.0)
            ones_row = cpool.tile([1, P], f32r)  # K=1 fold lhsT
            nc.vector.tensor_copy(ones_row, ones_row_f)

            # ---- load inputs (d on partitions) ----
            xTs = []
            for k in range(KC):
                xt = bpool.tile([P, BL], f32r, name=f"xT{k}")
                nc.sync.dma_start(xt, xT_d[k * P : (k + 1) * P, :])
                xTs.append(xt)
            cTs = []
            for k in range(KC):
                ct = bpool.tile([P, K], f32r, name=f"cT{k}")
                nc.sync.dma_start(ct, cT_d[k * P : (k + 1) * P, :])
                cTs.append(ct)

            # ---- squares (ACT), fp32r out ----
            xsqs = []
            for k in range(KC):
                xsq = sqpool.tile([P, BL], f32r, tag="xsq")
                nc.scalar.square(xsq, xTs[k].bitcast(f32))
                xsqs.append(xsq)
            csqs = []
            for k in range(KC):
                csq = sqpool.tile([P, K], f32r, tag="csq")
                nc.scalar.square(csq, cTs[k].bitcast(f32))
                csqs.append(csq)

            # ---- x2: (128, 2*MT), columns 2i/2i+1 both = ||x||^2 of m-tile i
            x2_ps = pprep.tile([P, 2 * MT], f32, tag="prep")
            for i in range(MT):
                for k in range(KC):
                    nc.tensor.matmul(
                        x2_ps[:, 2 * i : 2 * i + 2],
                        lhsT=xsqs[k][:, i * P : (i + 1) * P],
                        rhs=ones_col,
                        start=(k == 0),
                        stop=(k == KC - 1),
                    )
            x2c = cpool.tile([P, 2 * MT], f32)
            nc.vector.tensor_copy(x2c, x2_ps)

            # ---- c2 row (1, K), scaled by -0.5 ----
            c2_ps = pprep.tile([1, K], f32, tag="prep2")
            for h in range(JH):
                for k in range(KC):
                    nc.tensor.matmul(
                        c2_ps[0:1, h * NJ : (h + 1) * NJ],
                        lhsT=ones_col[:, 0:1],
                        rhs=csqs[k][:, h * NJ : (h + 1) * NJ],
                        start=(k == 0),
                        stop=(k == KC - 1),
                    )
            c2m = cpool.tile([1, K], f32r)
            nc.scalar.activation(c2m, c2_ps, AF.Copy, scale=-0.5)

            # ---- main: psum = x.c^T - c2/2 ; dist = Sqrt(-2 psum + x2) ----
            numbuf = bpool.tile([P, MT * K], f32)  # 64 KB/partition
            for i in range(MT):
                for h in range(JH):
                    ps = pmm.tile([P, NJ], f32, tag="mm")
                    for k in range(KC):
                        nc.tensor.matmul(
                            ps,
                            lhsT=xTs[k][:, i * P : (i + 1) * P],
                            rhs=cTs[k][:, h * NJ : (h + 1) * NJ],
                            start=(k == 0),
                            stop=False,
                        )
                    nc.tensor.matmul(
                        ps,
                        lhsT=ones_row,
                        rhs=c2m[0:1, h * NJ : (h + 1) * NJ],
                        start=False,
                        stop=True,
                    )
                    nc.scalar.activation(
                        numbuf[:, i * K + h * NJ : i * K + (h + 1) * NJ],
                        ps,
                        AF.Sqrt,
                        bias=x2c[:, 2 * i : 2 * i + 1],
                        scale=-2.0,
                    )

            # ---- num = 1/(1+dist) in place; fp32r copy feeds the sum ----
            sum_ps = pprep.tile([1, NJ], f32, tag="prep3")
            for s in range(NSL):
                sl = numbuf[:, s * SW : (s + 1) * SW]
                nc.vector.tensor_scalar_add(sl, sl, 1.0)
                nc.vector.reciprocal_approx_fast(sl, sl)
                nr = nrpool.tile([P, SW], f32r, tag="nr")
                nc.vector.tensor_copy(nr, sl)
                for t in range(SW // NJ):
                    nc.tensor.matmul(
                        sum_ps,
                        lhsT=ones_col[:, 0:1],
                        rhs=nr[:, t * NJ : (t + 1) * NJ],
                        start=(s == 0 and t == 0),
                        stop=(s == NSL - 1 and t == SW // NJ - 1),
                    )
            lsum = cpool.tile([1, 1], f32)
            nc.vector.reduce_sum(lsum, sum_ps, axis=mybir.AxisListType.X)

            # ---- AllReduce the scalar, then inv broadcast ----
            cc_in = dpool.tile([1, 1], f32)
            cc_out = dpool.tile([1, 1], f32, addr_space="Shared")
            nc.sync.dma_start(cc_in, lsum)
            nc.gpsimd.collective_compute(
                "AllReduce",
                mybir.AluOpType.add,
                replica_groups=[list(range(N_CORES))],
                ins=[cc_in.opt()],
                outs=[cc_out.opt()],
            )
            total = cpool.tile([1, 1], f32)
            nc.sync.dma_start(total, cc_out)
            inv = cpool.tile([1, 1], f32)
            nc.vector.reciprocal(inv, total)
            inv_ps = pprep.tile([P, 1], f32, tag="prep4")
            nc.tensor.matmul(inv_ps, lhsT=ones_row_f, rhs=inv, start=True, stop=True)
            invb = cpool.tile([P, 1], f32)
            nc.vector.tensor_copy(invb, inv_ps)

            # ---- scale + store ----
            for s in range(NSL):
                sl = numbuf[:, s * SW : (s + 1) * SW]
                nc.vector.tensor_scalar_mul(sl, sl, invb)
                nm = SW // K  # m-tiles per slice (2)
                i0 = s * nm
                dst = out_d[i0 * P : (i0 + nm) * P, :].rearrange(
                    "(f p) c -> p f c", p=P
                )
                src = numbuf[:, i0 * K : (i0 + nm) * K].rearrange(
                    "p (f c) -> p f c", f=nm
                )
                nc.sync.dma_start(dst, src)

    nc.finalize()
    return nc


def _get_bass():
    key = "nc"
    if key not in _CACHE:
        _CACHE[key] = _build_bass()
    return _CACHE[key]


def kernel(x: np.ndarray, clusters: np.ndarray) -> np.ndarray:
    from concourse.bass_utils import run_bass_kernel_spmd

    x = np.asarray(x, dtype=np.float32)
    clusters = np.asarray(clusters, dtype=np.float32)
    assert x.shape == (B, D) and clusters.shape == (K, D)

    cT = _round_f32r(np.ascontiguousarray(clusters.T))  # (D, K)
    in_maps = []
    for c in range(N_CORES):
        xT_c = _round_f32r(np.ascontiguousarray(x[c * BL : (c + 1) * BL].T))
        in_maps.append({"xT": xT_c, "cT": cT})

    nc = _get_bass()
    res = run_bass_kernel_spmd(nc, in_maps, core_ids=list(range(N_CORES)))
    return np.concatenate([r["out"] for r in res.results], axis=0)
